# revision 27
# baseline (speedup 1.0000x reference)
"""Trainium2 Bass kernel for nn_EquivarientScalar (segment_reduce).

Computation (reference): 2 stacked GatedEquivariant layers over N=100000
atoms (pointwise per atom), then sc = s @ out_w + out_b and a masked
segment-sum y[b] = sum_n sc[n] * batch_mask[b, n].

Strategy (~1.9x faster than the previous 190us bf16 kernel on HW):
  - Everything bf16 (inputs, weights, intermediates); matmul accum fp32
    in PSUM. 12800 atoms/core: 24 blocks of 512 + 1 tail block whose ops
    run at width TBLK.
  - Layer-2 scalar path folded on host: Wp = a2w_s0 @ a1w_s1.
  - Five-engine load balance (per-core busy, CoreSim): ACT ~88us
    (squares of v2 c0/c1 from PSUM, silu1/silu2, ~2/5 of the g evacs),
    DVE ~86us (gates, custom SQA' = c2^2 + sq1 which is the only
    PSUM-side norm op, ~3/5 of the g evacs, sc/y), Pool/GPSIMD ~62us
    (the whole SBUF-side norm chain: q = SQA'+sq0 add, u16 bit-trick
    rsqrt seed, n2 = q*y0 mult -- measured ~0.83ns/elem, far cheaper
    than the spec's 0.42-efficiency claim), PE ~75us, DMA ~55us.
  - rsqrt with NO Newton step: n2 = q * bf16_bits(MAGIC16 - bits(q)/2),
    with MAGIC16 fitted end-to-end against the reference (rel err
    3.7e-3 vs the 2e-2 gate). Seed runs as a TensorScalarPtr in 4x DVE
    perf mode when on DVE; custom DVE ops never get perf modes, stock
    TT/TSP do (2x needs all-2-byte packed operands; PSUM fp32 reads
    disqualify -- and DVE may read only ONE operand from PSUM).
  - Latency engineering (this, not engine work, bound the old kernel):
    xt/mask DMAs split and prefetched PREFETCH iterations ahead; stages
    emitted oldest-first (tail|mid2|mid|front) so resolved work sits at
    in-order queue heads; v13 matmuls hoisted before the silu/evac
    round-trips; deep offsets (2,4,6) give every cross-engine hop >1
    iteration of slack. Score metric is the reps-marginal (steady state),
    so pipeline ramp is amortized and pair-groups beat single blocks.
  - Segment reduce on-chip: sc columns via h2-chunk stationary matmuls,
    y += maskT_chunk^T @ sc per 128 atoms, mask in bf16 (0/1 exact).
    Host sums the 8 per-core partial y vectors.
"""

import os
import sys

for _p in ("/opt/trn_rl_repo", "/root/.axon_site/_ro/trn_rl_repo"):
    if os.path.isdir(_p) and _p not in sys.path:
        sys.path.insert(0, _p)

os.environ.setdefault("BASS_NEVER_TRACE", "1")  # no NTFF hook in this axon build

import numpy as np

import concourse.bass as bass
import concourse.tile as tile
from concourse import bacc, mybir
from concourse import dve_ops as _dve_ops
from concourse.alu_op_type import AluOpType
from concourse.bass_utils import run_bass_kernel_spmd
from concourse.dve_ops import OPS as _DVE_OPS
from concourse.dve_ops import _CUSTOM_DVE_ROW_BASE, _SUB_OPCODE_FOR_NAME, DveOp
from concourse.dve_spec import C0 as _C0
from concourse.dve_spec import C1 as _C1
from concourse.dve_spec import Spec as _Spec
from concourse.dve_spec import Src0 as _Src0
from concourse.dve_spec import Src1 as _Src1
from concourse.dve_spec import lower as _dve_lower
from concourse.dve_spec import sq as _sq
from concourse.dve_uop import DveOpSpec as _DveOpSpec

N_CORES = 8
NA_FULL = 100000
NA_CORE = NA_FULL // N_CORES   # 12500
BLK = 512
TBLK = 512   # tail block active width (256 measured slower on HW: keep 512)
NBLK = 25
NA = NBLK * BLK
F = 128

F32 = mybir.dt.float32
BF16 = mybir.dt.bfloat16
U32 = mybir.dt.uint32
U16 = mybir.dt.uint16
AF = mybir.ActivationFunctionType

W_NAMES = ["w1_0", "w2_0", "w2_1", "a1w_s0", "a1w_n0", "a1w_n1",
           "a2w_g0", "wp"]

# rsqrt magic seed (computed via u32 value-casts on Pool) + one fused
# Newton-ish stage on DVE. Constants fitted offline (baseline-validated):
# wide-range fp32 max rel err ~1e-3; q=0 -> 0 (no NaN).
MAGIC_F = 1596013007.0
SQ1_C0, SQ1_C1 = 1.6695484, 0.688087555  # n2 = (q*y0)*(C0 - C1*q*y0^2)
# bf16/u16 variant of the same trick, no Newton: n2 = q * bf16cast(MAGIC16
# - 0.5*bits(q)). MAGIC16 fitted on the end-to-end pipeline (see fit log).
MAGIC16 = 24373.0

_last_results = None
_last_nc = None
_last_in_maps = None
ABLATE = "full"  # timing ablations: full | no_scy | no_gate | no_norm | no_mid
GATE_MODE = "bcast"  # bcast: one 1536-wide op w/ zero-stride AP; planes: 3 ops
OFFSETS = (2, 4, 6)  # software-pipeline stage offsets (mid1, mid2, tail)
PREFETCH = 3         # xt DMA issued this many loop steps before front uses it
SEED_ENGINE = "pool"  # engine for the u16 bit-trick seed
N2M_ENGINE = "pool"   # engine for n2 = q*y0
G_EVAC_DVE_MOD = (3, 5)  # g-evac on DVE for (b*a) % m < a of blocks (balance)
NEWTON = False   # True: fp32 seed + custom-DVE Newton (accurate, slow)
Q01_ENGINE = "pool"  # sq0+sq1 add: pool (idle Q7 engine) | dve
# DVE can read only ONE operand from PSUM (one read port), so the gate
# cannot take both v13 and a2g from PSUM; a2g goes through the ACT evac.
GATE_G_FROM_PSUM = False


def _ref_sqa(in0, in1, s0, s1, imm2):
    return (in0.astype(np.float32) * in0 + in1).astype(np.float32)


def _ref_sqrt_fin(in0, in1, s0, s1, imm2):
    qy = (in0 * in1).astype(np.float32)
    return (qy * (np.float32(s0) - np.float32(s1) * (qy * in1))).astype(np.float32)


def _register_ops():
    by_name = {op.name: op for op in _DVE_OPS}
    if "SQA_ANT" in _SUB_OPCODE_FOR_NAME and "SQRT_FIN_ANT" in _SUB_OPCODE_FOR_NAME:
        return by_name["SQA_ANT"], by_name["SQRT_FIN_ANT"]

    def make(name, body, ref):
        if name in _SUB_OPCODE_FOR_NAME:
            return by_name[name]
        op = DveOp(name, _Spec(body=body, reference=ref), subdim=False,
                   uops_sha={})
        opcode = _CUSTOM_DVE_ROW_BASE + len(_DVE_OPS)
        for ver in ("v3", "v4"):
            try:
                spec = _DveOpSpec(name=name, opcode=opcode,
                                  uops=_dve_lower(op.spec, ver=ver),
                                  rd1_en=_dve_ops.has_src1(op.spec))
                op.uops_sha[ver] = spec.sha(ver)
            except Exception:
                pass
        _SUB_OPCODE_FOR_NAME[name] = opcode
        _DVE_OPS.append(op)
        return op

    # q = c2^2 + q01        [in0 = v2_c2 (PSUM), in1 = q01 (SBUF)]
    sqa = make("SQA_ANT", _sq(_Src0) + _Src1, _ref_sqa)
    # n2 = (q*y0) * (C0 - C1 * (q*y0) * y0)   [in0 = q, in1 = y0 seed]
    _qy = _Src0 * _Src1
    fin = make("SQRT_FIN_ANT",
               _qy * (_C0 - _C1 * (_qy * _Src1)),
               _ref_sqrt_fin)
    return sqa, fin


def _build(b_fold: float, reps: int = 1, trace_sim: bool = False):
    # reps > 1 repeats the whole computation inside one NEFF (timing only --
    # y then accumulates reps copies; used to subtract host/tunnel overhead).
    OP_SQA, OP_FIN = _register_ops()
    nc = bacc.Bacc("TRN2", target_bir_lowering=False)

    # packed input, partition-major: row f = [s 512 | v 3*512 | mask 4*128]
    xp = nc.dram_tensor("xpack", (F, NBLK, 5 * BLK), BF16, kind="ExternalInput")
    wd = {n: nc.dram_tensor(n, (F, F), BF16, kind="ExternalInput") for n in W_NAMES}
    # (F, 2): col 0 = a2w_s1 @ out_w, col 1 = zeros
    wf = nc.dram_tensor("w_fold", (F, 2), BF16, kind="ExternalInput")
    a1b0 = nc.dram_tensor("a1b0", (F, 1), F32, kind="ExternalInput")
    b2p = nc.dram_tensor("b2p", (F, 1), F32, kind="ExternalInput")
    a2bg = nc.dram_tensor("a2bg", (F, 1), F32, kind="ExternalInput")
    bfold = nc.dram_tensor("bfold", (F, 1), F32, kind="ExternalInput")
    y = nc.dram_tensor("y", (F, 1), F32, kind="ExternalOutput")

    # 12 pairs + tail block
    groups = [[2 * i, 2 * i + 1] for i in range(12)] + [[24]]

    with tile.TileContext(nc, trace_sim=trace_sim) as tc:
        # PSUM budget (8 banks): v3 tag 3 banks x2 bufs + "a" tag 1 bank x2.
        with tc.tile_pool(name="wpool", bufs=1) as wp, \
             tc.tile_pool(name="io", bufs=6) as io, \
             tc.tile_pool(name="iom", bufs=10) as iom, \
             tc.tile_pool(name="work", bufs=2) as wk, \
             tc.tile_pool(name="workx", bufs=7) as wkx, \
             tc.tile_pool(name="workv", bufs=6) as wkv, \
             tc.tile_pool(name="psv", bufs=2, space="PSUM") as psv, \
             tc.tile_pool(name="psa", bufs=2, space="PSUM") as psa:

            wt = {}
            for n in W_NAMES:
                wt[n] = wp.tile([F, F], BF16, name=n, tag=n)
                nc.sync.dma_start(out=wt[n], in_=wd[n][:, :])
            wft = wp.tile([F, 2], BF16, tag="wf")
            nc.sync.dma_start(out=wft, in_=wf[:, :])
            bt = {}
            for n, d in [("a1b0", a1b0), ("b2p", b2p), ("a2bg", a2bg),
                         ("bfold", bfold)]:
                bt[n] = wp.tile([F, 1], F32, name=n, tag=n)
                nc.sync.dma_start(out=bt[n], in_=d[:, :])

            y_sb = wp.tile([F, 1], F32, tag="y_sb")
            nc.vector.memset(y_sb, 0.0)

            def norm_front(xts_or_vouts, nh, tag, from_xt, W=BLK):
                """v-matmuls + squares for nh blocks; returns (v3s, q, n2P).
                Emits: per h: 1 matmul (N=1536) + 1 ACT square (c01) +
                1 DVE SQA; per group: 1 Pool add, 1 Pool seed, 1 DVE fin."""
                w = wt["w2_0"] if tag == "l1" else wt["w2_1"]
                sqP = (None if ABLATE in ("no_norm", "pe_only")
                       else wk.tile([F, nh, 2, BLK], BF16, tag=f"sqP_{tag}"))
                v3s = []
                for h in range(nh):
                    v3 = psv.tile([F, 3, BLK], F32, tag="v3")
                    for c in range(3):
                        if from_xt:
                            rhs = xts_or_vouts[:, h,
                                               (1 + c) * W:(2 + c) * W]
                        else:
                            rhs = xts_or_vouts[h][:, c, 0:W]
                        nc.tensor.matmul(v3[:, c, 0:W], w, rhs)
                    if ABLATE not in ("no_norm", "pe_only"):
                        nc.scalar.activation(out=sqP[:, h, :, 0:W],
                                             in_=v3[:, 0:2, 0:W],
                                             func=AF.Square)
                    v3s.append(v3)
                if ABLATE in ("no_norm", "pe_only"):
                    n2P = wkx.tile([F, nh, BLK], BF16, tag=f"n2P_{tag}")
                    nc.vector.memset(n2P, 0.5)
                    return v3s, n2P
                if NEWTON:
                    q01P = wk.tile([F, nh, BLK], BF16, tag=f"q01P_{tag}")
                    nc.vector.tensor_tensor(out=q01P, in0=sqP[:, :, 0, :],
                                            in1=sqP[:, :, 1, :],
                                            op=AluOpType.add)
                    q = wk.tile([F, nh, BLK], F32, tag=f"q_{tag}")
                    for h in range(nh):
                        nc.vector._custom_dve(OP_SQA, out=q[:, h, :],
                                              in0=v3s[h][:, 2, :],
                                              in1=q01P[:, h, :])
                    sd = wk.tile([F, nh, BLK], F32, tag=f"sd_{tag}")
                    seed_eng = nc.vector if SEED_ENGINE == "dve" else nc.gpsimd
                    seed_eng.tensor_scalar(out=sd.bitcast(U32),
                                           in0=q.bitcast(U32),
                                           scalar1=-0.5, scalar2=MAGIC_F,
                                           op0=AluOpType.mult, op1=AluOpType.add)
                    n2P = wkx.tile([F, nh, BLK], BF16, tag=f"n2P_{tag}")
                    nc.vector._custom_dve(OP_FIN, out=n2P, in0=q, in1=sd,
                                          s0=SQ1_C0, s1=SQ1_C1)
                    return v3s, n2P
                # bf16 path, no Newton: SQA' = c2^2 + sq1 runs first so the
                # PSUM v3 tile frees without waiting on the Pool engine; the
                # final 3-way add then runs on the otherwise idle Pool, and
                # seed (TSP 4x) + n2 = q*y0 (TT 2x) finish on DVE.
                qa = wk.tile([F, nh, BLK], BF16, tag=f"qa_{tag}")
                for h in range(nh):
                    nc.vector._custom_dve(OP_SQA, out=qa[:, h, 0:W],
                                          in0=v3s[h][:, 2, 0:W],
                                          in1=sqP[:, h, 1, 0:W])
                q = wk.tile([F, nh, BLK], BF16, tag=f"q_{tag}")
                q_eng = nc.gpsimd if Q01_ENGINE == "pool" else nc.vector
                q_eng.tensor_tensor(out=q[:, :, 0:W], in0=qa[:, :, 0:W],
                                    in1=sqP[:, :, 0, 0:W],
                                    op=AluOpType.add)
                sd = wk.tile([F, nh, BLK], BF16, tag=f"sd_{tag}")
                seed_eng = nc.gpsimd if SEED_ENGINE == "pool" else nc.vector
                seed_eng.tensor_scalar(out=sd.bitcast(U16)[:, :, 0:W],
                                       in0=q.bitcast(U16)[:, :, 0:W],
                                       scalar1=-0.5, scalar2=MAGIC16,
                                       op0=AluOpType.mult, op1=AluOpType.add)
                n2P = wkx.tile([F, nh, BLK], BF16, tag=f"n2P_{tag}")
                n2_eng = nc.gpsimd if N2M_ENGINE == "pool" else nc.vector
                n2_eng.tensor_tensor(out=n2P[:, :, 0:W], in0=q[:, :, 0:W],
                                     in1=sd[:, :, 0:W],
                                     op=AluOpType.mult)
                return v3s, n2P

            # Manual software pipeline: dma(i) is prefetched PF iterations
            # before front(i) consumes it, and per loop step the stages are
            # emitted oldest-first (tail | mid2 | mid | front) so work whose
            # inputs are already resolved sits at each engine's queue head.
            ctxs = {}

            def st_dma(it):
                hs = groups[it % len(groups)]
                nh = len(hs)
                b0 = hs[0]
                W = TBLK if b0 == NBLK - 1 else BLK
                xt = io.tile([F, nh, 4 * BLK], BF16, tag="xt")
                xm = iom.tile([F, nh, BLK], BF16, tag="xm")
                with tc.high_priority(offset=110):
                    nc.sync.dma_start(out=xt[:, :, 0:4 * W],
                                      in_=xp[:, b0:b0 + nh, 0:4 * W])
                    nc.sync.dma_start(out=xm[:, :, 0:W],
                                      in_=xp[:, b0:b0 + nh, 4 * W:5 * W])
                ctxs[it] = {"nh": nh, "W": W, "xt": xt, "xm": xm}

            def st_front(it):
                cx = ctxs[it]
                _, n2P1 = norm_front(cx["xt"], cx["nh"], "l1", True,
                                     W=cx["W"])
                cx["n2P1"] = n2P1

            def st_mid(it):
                cx = ctxs[it]
                hs = groups[it % len(groups)]
                nh, xt, n2P1 = cx["nh"], cx["xt"], cx["n2P1"]
                W = cx["W"]
                if ABLATE == "no_mid":
                    h1P = wkx.tile([F, nh, BLK], BF16, tag="h1P")
                    nc.vector.memset(h1P, 0.5)
                    cx["h1P"], cx["vouts"] = h1P, None
                    return
                h1P = wkx.tile([F, nh, BLK], BF16, tag="h1P")
                gP = (None if GATE_G_FROM_PSUM
                      else wk.tile([F, nh, BLK], BF16, tag="gP"))
                vouts = []
                for h in range(nh):
                    # v1 matmuls first: they only need xt, so the PE runs
                    # them while ACT is busy with the silu/evac round-trips.
                    v13 = psv.tile([F, 3, BLK], F32, tag="v3")
                    for c in range(3):
                        nc.tensor.matmul(v13[:, c, 0:W], wt["w1_0"],
                                         xt[:, h, (1 + c) * W:(2 + c) * W])

                    a1 = psa.tile([F, BLK], F32, tag="a")
                    nc.tensor.matmul(a1[:, 0:W], wt["a1w_s0"], xt[:, h, 0:W],
                                     start=True, stop=False)
                    nc.tensor.matmul(a1[:, 0:W], wt["a1w_n0"],
                                     n2P1[:, h, 0:W],
                                     start=False, stop=True)
                    nc.scalar.activation(out=h1P[:, h, 0:W], in_=a1[:, 0:W],
                                         func=AF.Silu, bias=bt["a1b0"])

                    a2g = psa.tile([F, BLK], F32, tag="a")
                    nc.tensor.matmul(a2g[:, 0:W], wt["a2w_g0"],
                                     h1P[:, h, 0:W])
                    if GATE_G_FROM_PSUM:
                        g = a2g[:, 0:W]
                    else:
                        a_, m_ = G_EVAC_DVE_MOD
                        if (hs[h] * a_) % m_ < a_:
                            nc.vector.tensor_scalar(
                                out=gP[:, h, 0:W], in0=a2g[:, 0:W],
                                scalar1=bt["a2bg"], scalar2=None,
                                op0=AluOpType.add)
                        else:
                            nc.scalar.activation(out=gP[:, h, 0:W],
                                                 in_=a2g[:, 0:W],
                                                 func=AF.Identity,
                                                 bias=bt["a2bg"])
                        g = gP[:, h, 0:W]

                    g_bc3 = bass.AP(tensor=g.tensor, offset=g.offset,
                                    ap=[g.ap[0], [0, 3], g.ap[1]])
                    vout = wkv.tile([F, 3, BLK], BF16, tag="vout")
                    if ABLATE in ("no_gate", "pe_only"):
                        nc.vector.memset(vout, 0.5)
                    elif GATE_MODE == "bcast":
                        nc.vector.tensor_tensor(out=vout[:, :, 0:W],
                                                in0=v13[:, :, 0:W],
                                                in1=g_bc3,
                                                op=AluOpType.mult)
                    else:
                        for c in range(3):
                            nc.vector.tensor_tensor(out=vout[:, c, 0:W],
                                                    in0=v13[:, c, 0:W],
                                                    in1=g,
                                                    op=AluOpType.mult)
                    vouts.append(vout)

                cx["h1P"], cx["vouts"] = h1P, vouts

            def st_mid2(it):
                cx = ctxs[it]
                nh = cx["nh"]
                if ABLATE == "no_mid":
                    n2P2 = wkx.tile([F, nh, BLK], BF16, tag="n2P_l2")
                    nc.vector.memset(n2P2, 0.5)
                    cx["n2P2"] = n2P2
                    return
                _, n2P2 = norm_front(cx.pop("vouts"), nh, "l2", False,
                                     W=cx["W"])
                cx["n2P2"] = n2P2

            def st_tail(it):
                cx = ctxs.pop(it)
                nh, xm, h1P, n2P2 = cx["nh"], cx["xm"], cx["h1P"], cx["n2P2"]
                W = cx["W"]
                h2P = wk.tile([F, nh, BLK], BF16, tag="h2P")
                for h in range(nh):
                    a1b_ = psa.tile([F, BLK], F32, tag="a")
                    nc.tensor.matmul(a1b_[:, 0:W], wt["wp"], h1P[:, h, 0:W],
                                     start=True, stop=False)
                    nc.tensor.matmul(a1b_[:, 0:W], wt["a1w_n1"],
                                     n2P2[:, h, 0:W],
                                     start=False, stop=True)
                    nc.scalar.activation(out=h2P[:, h, 0:W],
                                         in_=a1b_[:, 0:W],
                                         func=AF.Silu, bias=bt["b2p"])

                if ABLATE in ("no_scy",):
                    return
                sc_ps = psa.tile([F, nh, BLK // F, 2], F32, tag="a")
                for h in range(nh):
                    for k in range(W // F):
                        nc.tensor.matmul(sc_ps[:, h, k, :],
                                         h2P[:, h, k * F:(k + 1) * F], wft)
                sc_sb = wk.tile([F, nh, BLK // F, 2], BF16, tag="sc_sb")
                nc.vector.tensor_scalar(out=sc_sb[:, :, 0:W // F, :],
                                        in0=sc_ps[:, :, 0:W // F, :],
                                        scalar1=bt["bfold"],
                                        scalar2=None, op0=AluOpType.add)
                y_ps = psa.tile([F, 2], F32, tag="a")
                nmm = 0
                nmm_tot = nh * (W // F)
                for h in range(nh):
                    mT_t = xm[:, h, 0:W].rearrange("p (k g) -> p k g",
                                                   k=W // F)
                    for k in range(W // F):
                        nc.tensor.matmul(y_ps, mT_t[:, k, :],
                                         sc_sb[:, h, k, :],
                                         start=(nmm == 0),
                                         stop=(nmm == nmm_tot - 1),
                                         skip_group_check=True)
                        nmm += 1
                nc.vector.tensor_tensor(out=y_sb, in0=y_sb, in1=y_ps[:, 0:1],
                                        op=AluOpType.add)

            o1, o2, o3 = OFFSETS
            total = len(groups) * reps
            for it in range(total + PREFETCH + o3):
                if it < total:
                    st_dma(it)
                if 0 <= it - PREFETCH - o3 < total:
                    st_tail(it - PREFETCH - o3)
                if 0 <= it - PREFETCH - o2 < total:
                    st_mid2(it - PREFETCH - o2)
                if 0 <= it - PREFETCH - o1 < total:
                    st_mid(it - PREFETCH - o1)
                if 0 <= it - PREFETCH < total:
                    st_front(it - PREFETCH)

            nc.sync.dma_start(out=y[:, :], in_=y_sb)

    nc.finalize()
    return nc


def kernel(s, v, r, batch_mask, w1, w2, a1w, a1b, a2w, a2b, out_w, out_b):
    global _last_results, GATE_G_FROM_PSUM
    del r  # unused by the reference computation

    # The PSUM-direct gate skips the +a2bg bias (zero for this model init);
    # fall back to the ACT-evac path if a nonzero gate bias ever shows up.
    if np.any(np.asarray(a2b)[0, F:] != 0):
        GATE_G_FROM_PSUM = False

    s = np.ascontiguousarray(np.asarray(s, dtype=np.float32)).reshape(NA_FULL, F)
    v = np.ascontiguousarray(np.asarray(v, dtype=np.float32)).reshape(NA_FULL, 3, F)
    batch_mask = np.ascontiguousarray(
        np.asarray(batch_mask, dtype=np.float32)).reshape(F, NA_FULL)
    w1 = np.asarray(w1, dtype=np.float64)
    w2 = np.asarray(w2, dtype=np.float64)
    a1w = np.asarray(a1w, dtype=np.float64)
    a1b = np.asarray(a1b, dtype=np.float64)
    a2w = np.asarray(a2w, dtype=np.float64)
    a2b = np.asarray(a2b, dtype=np.float64)
    out_w = np.asarray(out_w, dtype=np.float64)
    out_b = np.asarray(out_b, dtype=np.float64)
    assert w1.shape == (2, F, F), "kernel is specialized to L=2"

    bf16 = mybir.dt.np(BF16)

    # folded final projection: sc = h2 @ (a2w_s1 @ out_w) + b_fold
    w_fold = np.zeros((F, 2), dtype=np.float64)
    w_fold[:, 0:1] = a2w[1][:, :F] @ out_w
    b_fold = float(a2b[1][:F] @ out_w[:, 0] + out_b[0])

    # folded layer-2 scalar path: a1w_s1^T s1 = (a2w_s0 @ a1w_s1)^T h1 + const
    wp = a2w[0][:, :F] @ a1w[1][:F, :]
    b2p = a1b[1] + a1w[1][:F, :].T @ a2b[0][:F]

    weights = {
        "w1_0": w1[0], "w2_0": w2[0], "w2_1": w2[1],
        "a1w_s0": a1w[0][:F, :], "a1w_n0": a1w[0][F:, :],
        "a1w_n1": a1w[1][F:, :], "a2w_g0": a2w[0][:, F:], "wp": wp,
    }
    weights = {k: np.ascontiguousarray(a, dtype=bf16)
               for k, a in weights.items()}
    w_fold16 = np.ascontiguousarray(w_fold, dtype=bf16)
    bias_cols = {
        "bfold": np.full((F, 1), b_fold, dtype=np.float32),
        "a1b0": np.ascontiguousarray(a1b[0].reshape(F, 1), dtype=np.float32),
        "b2p": np.ascontiguousarray(b2p.reshape(F, 1), dtype=np.float32),
        "a2bg": np.ascontiguousarray(a2b[0][F:].reshape(F, 1),
                                     dtype=np.float32),
    }

    in_maps = []
    for c in range(N_CORES):
        sl = slice(c * NA_CORE, (c + 1) * NA_CORE)
        sT = np.zeros((F, NA), dtype=np.float32)
        sT[:, :NA_CORE] = s[sl].T
        vT = np.zeros((F, 3, NA), dtype=np.float32)
        vT[:, :, :NA_CORE] = v[sl].transpose(2, 1, 0)
        mT = np.zeros((NA, F), dtype=np.float32)
        mT[:NA_CORE] = batch_mask[:, sl].T
        xp = np.zeros((F, NBLK, 5 * BLK), dtype=np.float32)
        NF = (NBLK - 1) * BLK
        xp[:, :-1, 0:BLK] = sT[:, :NF].reshape(F, NBLK - 1, BLK)
        xp[:, :-1, BLK:4 * BLK] = (
            vT[:, :, :NF].reshape(F, 3, NBLK - 1, BLK).transpose(0, 2, 1, 3)
            .reshape(F, NBLK - 1, 3 * BLK))
        xp[:, :-1, 4 * BLK:] = (
            mT[:NF].reshape(NBLK - 1, BLK // F, F, F).transpose(2, 0, 1, 3)
            .reshape(F, NBLK - 1, BLK))
        # tail block packed tight at TBLK: [s | v0 v1 v2 | mask chunks]
        xp[:, -1, 0:TBLK] = sT[:, NF:NF + TBLK]
        xp[:, -1, TBLK:4 * TBLK] = (
            vT[:, :, NF:NF + TBLK].reshape(F, 3 * TBLK))
        xp[:, -1, 4 * TBLK:5 * TBLK] = (
            mT[NF:NF + TBLK].reshape(TBLK // F, F, F).transpose(1, 0, 2)
            .reshape(F, TBLK))
        m = {"xpack": np.ascontiguousarray(xp, dtype=bf16),
             "w_fold": w_fold16}
        m.update(weights)
        m.update(bias_cols)
        in_maps.append(m)

    nc = _build(b_fold)
    res = run_bass_kernel_spmd(nc, in_maps, core_ids=list(range(N_CORES)))
    global _last_nc, _last_in_maps
    _last_results, _last_nc, _last_in_maps = res, nc, in_maps

    yv = np.zeros((F, 1), dtype=np.float64)
    for c in range(N_CORES):
        yv += res.results[c]["y"].astype(np.float64)
    return yv.astype(np.float32)



# revision 28
# speedup vs baseline: 1.1219x; 1.1219x over previous
"""Trainium2 Bass kernel for nn_EquivarientScalar (segment_reduce).

Computation (reference): 2 stacked GatedEquivariant layers over N=100000
atoms (pointwise per atom), then sc = s @ out_w + out_b and a masked
segment-sum y[b] = sum_n sc[n] * batch_mask[b, n].

Strategy (~1.9x faster than the previous 190us bf16 kernel on HW):
  - Everything bf16 (inputs, weights, intermediates); matmul accum fp32
    in PSUM. 12800 atoms/core: 24 blocks of 512 + 1 tail block whose ops
    run at width TBLK.
  - Layer-2 scalar path folded on host: Wp = a2w_s0 @ a1w_s1.
  - Five-engine load balance (per-core busy, CoreSim): ACT ~88us
    (squares of v2 c0/c1 from PSUM, silu1/silu2, ~2/5 of the g evacs),
    DVE ~86us (gates, custom SQA' = c2^2 + sq1 which is the only
    PSUM-side norm op, ~3/5 of the g evacs, sc/y), Pool/GPSIMD ~62us
    (the whole SBUF-side norm chain: q = SQA'+sq0 add, u16 bit-trick
    rsqrt seed, n2 = q*y0 mult -- measured ~0.83ns/elem, far cheaper
    than the spec's 0.42-efficiency claim), PE ~75us, DMA ~55us.
  - rsqrt with NO Newton step: n2 = q * bf16_bits(MAGIC16 - bits(q)/2),
    with MAGIC16 fitted end-to-end against the reference (rel err
    3.7e-3 vs the 2e-2 gate). Seed runs as a TensorScalarPtr in 4x DVE
    perf mode when on DVE; custom DVE ops never get perf modes, stock
    TT/TSP do (2x needs all-2-byte packed operands; PSUM fp32 reads
    disqualify -- and DVE may read only ONE operand from PSUM).
  - Latency engineering (this, not engine work, bound the old kernel):
    xt/mask DMAs split and prefetched PREFETCH iterations ahead; stages
    emitted oldest-first (tail|mid2|mid|front) so resolved work sits at
    in-order queue heads; v13 matmuls hoisted before the silu/evac
    round-trips; deep offsets (2,4,6) give every cross-engine hop >1
    iteration of slack. Score metric is the reps-marginal (steady state),
    so pipeline ramp is amortized and pair-groups beat single blocks.
  - Segment reduce on-chip: sc columns via h2-chunk stationary matmuls,
    y += maskT_chunk^T @ sc per 128 atoms, mask in bf16 (0/1 exact).
    Host sums the 8 per-core partial y vectors.
"""

import os
import sys

for _p in ("/opt/trn_rl_repo", "/root/.axon_site/_ro/trn_rl_repo"):
    if os.path.isdir(_p) and _p not in sys.path:
        sys.path.insert(0, _p)

os.environ.setdefault("BASS_NEVER_TRACE", "1")  # no NTFF hook in this axon build

import numpy as np

import concourse.bass as bass
import concourse.tile as tile
from concourse import bacc, mybir
from concourse import dve_ops as _dve_ops
from concourse.alu_op_type import AluOpType
from concourse.bass_utils import run_bass_kernel_spmd
from concourse.dve_ops import OPS as _DVE_OPS
from concourse.dve_ops import _CUSTOM_DVE_ROW_BASE, _SUB_OPCODE_FOR_NAME, DveOp
from concourse.dve_spec import C0 as _C0
from concourse.dve_spec import C1 as _C1
from concourse.dve_spec import Spec as _Spec
from concourse.dve_spec import Src0 as _Src0
from concourse.dve_spec import Src1 as _Src1
from concourse.dve_spec import lower as _dve_lower
from concourse.dve_spec import sq as _sq
from concourse.dve_uop import DveOpSpec as _DveOpSpec

N_CORES = 8
NA_FULL = 100000
NA_CORE = NA_FULL // N_CORES   # 12500
BLK = 512
TBLK = 512   # tail block active width (256 measured slower on HW: keep 512)
NBLK = 25
NA = NBLK * BLK
F = 128

F32 = mybir.dt.float32
BF16 = mybir.dt.bfloat16
U32 = mybir.dt.uint32
U16 = mybir.dt.uint16
AF = mybir.ActivationFunctionType

W_NAMES = ["w1_0", "w2_0", "w2_1", "a1w_s0", "a1w_n0", "a1w_n1",
           "a2w_g0", "wp"]

# rsqrt magic seed (computed via u32 value-casts on Pool) + one fused
# Newton-ish stage on DVE. Constants fitted offline (baseline-validated):
# wide-range fp32 max rel err ~1e-3; q=0 -> 0 (no NaN).
MAGIC_F = 1596013007.0
SQ1_C0, SQ1_C1 = 1.6695484, 0.688087555  # n2 = (q*y0)*(C0 - C1*q*y0^2)
# bf16/u16 variant of the same trick, no Newton: n2 = q * bf16cast(MAGIC16
# - 0.5*bits(q)). MAGIC16 fitted on the end-to-end pipeline (see fit log).
MAGIC16 = 24373.0

_last_results = None
_last_nc = None
_last_in_maps = None
ABLATE = "full"  # timing ablations: full | no_scy | no_gate | no_norm | no_mid
GATE_MODE = "bcast"  # bcast: one 1536-wide op w/ zero-stride AP; planes: 3 ops
OFFSETS = (2, 4, 6)  # software-pipeline stage offsets (mid1, mid2, tail)
PREFETCH = 3         # xt DMA issued this many loop steps before front uses it
SEED_ENGINE = "pool"  # engine for the u16 bit-trick seed
N2M_ENGINE = "pool"   # engine for n2 = q*y0
G_EVAC_DVE_MOD = (3, 5)  # g-evac on DVE for (b*a) % m < a of blocks (balance)
NEWTON = False   # True: fp32 seed + custom-DVE Newton (accurate, slow)
Q01_ENGINE = "pool"  # sq0+sq1 add: pool (idle Q7 engine) | dve
# DVE can read only ONE operand from PSUM (one read port), so the gate
# cannot take both v13 and a2g from PSUM; a2g goes through the ACT evac.
GATE_G_FROM_PSUM = False


def _ref_sqa(in0, in1, s0, s1, imm2):
    return (in0.astype(np.float32) * in0 + in1).astype(np.float32)


def _ref_sqrt_fin(in0, in1, s0, s1, imm2):
    qy = (in0 * in1).astype(np.float32)
    return (qy * (np.float32(s0) - np.float32(s1) * (qy * in1))).astype(np.float32)


def _register_ops():
    by_name = {op.name: op for op in _DVE_OPS}
    if "SQA_ANT" in _SUB_OPCODE_FOR_NAME and "SQRT_FIN_ANT" in _SUB_OPCODE_FOR_NAME:
        return by_name["SQA_ANT"], by_name["SQRT_FIN_ANT"]

    def make(name, body, ref):
        if name in _SUB_OPCODE_FOR_NAME:
            return by_name[name]
        op = DveOp(name, _Spec(body=body, reference=ref), subdim=False,
                   uops_sha={})
        opcode = _CUSTOM_DVE_ROW_BASE + len(_DVE_OPS)
        for ver in ("v3", "v4"):
            try:
                spec = _DveOpSpec(name=name, opcode=opcode,
                                  uops=_dve_lower(op.spec, ver=ver),
                                  rd1_en=_dve_ops.has_src1(op.spec))
                op.uops_sha[ver] = spec.sha(ver)
            except Exception:
                pass
        _SUB_OPCODE_FOR_NAME[name] = opcode
        _DVE_OPS.append(op)
        return op

    # q = c2^2 + q01        [in0 = v2_c2 (PSUM), in1 = q01 (SBUF)]
    sqa = make("SQA_ANT", _sq(_Src0) + _Src1, _ref_sqa)
    # n2 = (q*y0) * (C0 - C1 * (q*y0) * y0)   [in0 = q, in1 = y0 seed]
    _qy = _Src0 * _Src1
    fin = make("SQRT_FIN_ANT",
               _qy * (_C0 - _C1 * (_qy * _Src1)),
               _ref_sqrt_fin)
    return sqa, fin


def _build(b_fold: float, reps: int = 1, trace_sim: bool = False):
    # reps > 1 repeats the whole computation inside one NEFF (timing only --
    # y then accumulates reps copies; used to subtract host/tunnel overhead).
    OP_SQA, OP_FIN = _register_ops()
    nc = bacc.Bacc("TRN2", target_bir_lowering=False)

    # packed input, partition-major: row f = [s 512 | v 3*512 | mask 4*128]
    xp = nc.dram_tensor("xpack", (F, NBLK, 5 * BLK), BF16, kind="ExternalInput")
    wd = {n: nc.dram_tensor(n, (F, F), BF16, kind="ExternalInput") for n in W_NAMES}
    # (F, 2): col 0 = a2w_s1 @ out_w, col 1 = zeros
    wf = nc.dram_tensor("w_fold", (F, 2), BF16, kind="ExternalInput")
    a1b0 = nc.dram_tensor("a1b0", (F, 1), F32, kind="ExternalInput")
    b2p = nc.dram_tensor("b2p", (F, 1), F32, kind="ExternalInput")
    a2bg = nc.dram_tensor("a2bg", (F, 1), F32, kind="ExternalInput")
    bfold = nc.dram_tensor("bfold", (F, 1), F32, kind="ExternalInput")
    y = nc.dram_tensor("y", (F, 1), F32, kind="ExternalOutput")

    # 12 pairs + tail block
    groups = [[2 * i, 2 * i + 1] for i in range(12)] + [[24]]

    with tile.TileContext(nc, trace_sim=trace_sim) as tc:
        # PSUM budget (8 banks): v3 tag 3 banks x2 bufs + "a" tag 1 bank x2.
        with tc.tile_pool(name="wpool", bufs=1) as wp, \
             tc.tile_pool(name="io", bufs=6) as io, \
             tc.tile_pool(name="iom", bufs=10) as iom, \
             tc.tile_pool(name="work", bufs=2) as wk, \
             tc.tile_pool(name="workx", bufs=7) as wkx, \
             tc.tile_pool(name="workv", bufs=6) as wkv, \
             tc.tile_pool(name="psv", bufs=2, space="PSUM") as psv, \
             tc.tile_pool(name="psa", bufs=2, space="PSUM") as psa:

            wt = {}
            for n in W_NAMES:
                wt[n] = wp.tile([F, F], BF16, name=n, tag=n)
                nc.sync.dma_start(out=wt[n], in_=wd[n][:, :])
            wft = wp.tile([F, 2], BF16, tag="wf")
            nc.sync.dma_start(out=wft, in_=wf[:, :])
            bt = {}
            for n, d in [("a1b0", a1b0), ("b2p", b2p), ("a2bg", a2bg),
                         ("bfold", bfold)]:
                bt[n] = wp.tile([F, 1], F32, name=n, tag=n)
                nc.sync.dma_start(out=bt[n], in_=d[:, :])

            y_sb = wp.tile([F, 1], F32, tag="y_sb")
            nc.vector.memset(y_sb, 0.0)

            def norm_front(xts_or_vouts, nh, tag, from_xt, W=BLK):
                """v-matmuls + squares for nh blocks; returns (v3s, q, n2P).
                Emits: per h: 1 matmul (N=1536) + 1 ACT square (c01) +
                1 DVE SQA; per group: 1 Pool add, 1 Pool seed, 1 DVE fin."""
                w = wt["w2_0"] if tag == "l1" else wt["w2_1"]
                sqP = (None if ABLATE in ("no_norm", "pe_only")
                       else wk.tile([F, nh, 2, BLK], BF16, tag=f"sqP_{tag}"))
                v3s = []
                for h in range(nh):
                    v3 = psv.tile([F, 3, BLK], F32, tag="v3")
                    for c in range(3):
                        if from_xt:
                            rhs = xts_or_vouts[:, h,
                                               (1 + c) * W:(2 + c) * W]
                        else:
                            rhs = xts_or_vouts[h][:, c, 0:W]
                        nc.tensor.matmul(v3[:, c, 0:W], w, rhs)
                    if ABLATE not in ("no_norm", "pe_only"):
                        nc.scalar.activation(out=sqP[:, h, :, 0:W],
                                             in_=v3[:, 0:2, 0:W],
                                             func=AF.Square)
                    v3s.append(v3)
                if ABLATE in ("no_norm", "pe_only"):
                    n2P = wkx.tile([F, nh, BLK], BF16, tag=f"n2P_{tag}")
                    nc.vector.memset(n2P, 0.5)
                    return v3s, n2P
                if NEWTON:
                    q01P = wk.tile([F, nh, BLK], BF16, tag=f"q01P_{tag}")
                    nc.vector.tensor_tensor(out=q01P, in0=sqP[:, :, 0, :],
                                            in1=sqP[:, :, 1, :],
                                            op=AluOpType.add)
                    q = wk.tile([F, nh, BLK], F32, tag=f"q_{tag}")
                    for h in range(nh):
                        nc.vector._custom_dve(OP_SQA, out=q[:, h, :],
                                              in0=v3s[h][:, 2, :],
                                              in1=q01P[:, h, :])
                    sd = wk.tile([F, nh, BLK], F32, tag=f"sd_{tag}")
                    seed_eng = nc.vector if SEED_ENGINE == "dve" else nc.gpsimd
                    seed_eng.tensor_scalar(out=sd.bitcast(U32),
                                           in0=q.bitcast(U32),
                                           scalar1=-0.5, scalar2=MAGIC_F,
                                           op0=AluOpType.mult, op1=AluOpType.add)
                    n2P = wkx.tile([F, nh, BLK], BF16, tag=f"n2P_{tag}")
                    nc.vector._custom_dve(OP_FIN, out=n2P, in0=q, in1=sd,
                                          s0=SQ1_C0, s1=SQ1_C1)
                    return v3s, n2P
                # bf16 path, no Newton: SQA' = c2^2 + sq1 runs first so the
                # PSUM v3 tile frees without waiting on the Pool engine; the
                # final 3-way add then runs on the otherwise idle Pool, and
                # seed (TSP 4x) + n2 = q*y0 (TT 2x) finish on DVE.
                qa = wk.tile([F, nh, BLK], BF16, tag=f"qa_{tag}")
                for h in range(nh):
                    nc.vector._custom_dve(OP_SQA, out=qa[:, h, 0:W],
                                          in0=v3s[h][:, 2, 0:W],
                                          in1=sqP[:, h, 1, 0:W])
                q = wk.tile([F, nh, BLK], BF16, tag=f"q_{tag}")
                q_eng = nc.gpsimd if Q01_ENGINE == "pool" else nc.vector
                q_eng.tensor_tensor(out=q[:, :, 0:W], in0=qa[:, :, 0:W],
                                    in1=sqP[:, :, 0, 0:W],
                                    op=AluOpType.add)
                sd = wk.tile([F, nh, BLK], BF16, tag=f"sd_{tag}")
                seed_eng = nc.gpsimd if SEED_ENGINE == "pool" else nc.vector
                seed_eng.tensor_scalar(out=sd.bitcast(U16)[:, :, 0:W],
                                       in0=q.bitcast(U16)[:, :, 0:W],
                                       scalar1=-0.5, scalar2=MAGIC16,
                                       op0=AluOpType.mult, op1=AluOpType.add)
                n2P = wkx.tile([F, nh, BLK], BF16, tag=f"n2P_{tag}")
                n2_eng = nc.gpsimd if N2M_ENGINE == "pool" else nc.vector
                n2_eng.tensor_tensor(out=n2P[:, :, 0:W], in0=q[:, :, 0:W],
                                     in1=sd[:, :, 0:W],
                                     op=AluOpType.mult)
                return v3s, n2P

            # Manual software pipeline: dma(i) is prefetched PF iterations
            # before front(i) consumes it, and per loop step the stages are
            # emitted oldest-first (tail | mid2 | mid | front) so work whose
            # inputs are already resolved sits at each engine's queue head.
            ctxs = {}

            def st_dma(it):
                hs = groups[it % len(groups)]
                nh = len(hs)
                b0 = hs[0]
                W = TBLK if b0 == NBLK - 1 else BLK
                xt = io.tile([F, nh, 4 * BLK], BF16, tag="xt")
                xm = iom.tile([F, nh, BLK], BF16, tag="xm")
                with tc.high_priority(offset=110):
                    nc.sync.dma_start(out=xt[:, :, 0:4 * W],
                                      in_=xp[:, b0:b0 + nh, 0:4 * W])
                    nc.sync.dma_start(out=xm[:, :, 0:W],
                                      in_=xp[:, b0:b0 + nh, 4 * W:5 * W])
                ctxs[it] = {"nh": nh, "W": W, "xt": xt, "xm": xm}

            def st_front(it):
                cx = ctxs[it]
                _, n2P1 = norm_front(cx["xt"], cx["nh"], "l1", True,
                                     W=cx["W"])
                cx["n2P1"] = n2P1

            def st_mid(it):
                cx = ctxs[it]
                hs = groups[it % len(groups)]
                nh, xt, n2P1 = cx["nh"], cx["xt"], cx["n2P1"]
                W = cx["W"]
                if ABLATE == "no_mid":
                    h1P = wkx.tile([F, nh, BLK], BF16, tag="h1P")
                    nc.vector.memset(h1P, 0.5)
                    cx["h1P"], cx["vouts"] = h1P, None
                    return
                h1P = wkx.tile([F, nh, BLK], BF16, tag="h1P")
                gP = (None if GATE_G_FROM_PSUM
                      else wk.tile([F, nh, BLK], BF16, tag="gP"))
                vouts = []
                for h in range(nh):
                    # v1 matmuls first: they only need xt, so the PE runs
                    # them while ACT is busy with the silu/evac round-trips.
                    v13 = psv.tile([F, 3, BLK], F32, tag="v3")
                    for c in range(3):
                        nc.tensor.matmul(v13[:, c, 0:W], wt["w1_0"],
                                         xt[:, h, (1 + c) * W:(2 + c) * W])

                    a1 = psa.tile([F, BLK], F32, tag="a")
                    nc.tensor.matmul(a1[:, 0:W], wt["a1w_s0"], xt[:, h, 0:W],
                                     start=True, stop=False)
                    nc.tensor.matmul(a1[:, 0:W], wt["a1w_n0"],
                                     n2P1[:, h, 0:W],
                                     start=False, stop=True)
                    nc.scalar.activation(out=h1P[:, h, 0:W], in_=a1[:, 0:W],
                                         func=AF.Silu, bias=bt["a1b0"])

                    a2g = psa.tile([F, BLK], F32, tag="a")
                    nc.tensor.matmul(a2g[:, 0:W], wt["a2w_g0"],
                                     h1P[:, h, 0:W])
                    if GATE_G_FROM_PSUM:
                        g = a2g[:, 0:W]
                    else:
                        a_, m_ = G_EVAC_DVE_MOD
                        if (hs[h] * a_) % m_ < a_:
                            nc.vector.tensor_scalar(
                                out=gP[:, h, 0:W], in0=a2g[:, 0:W],
                                scalar1=bt["a2bg"], scalar2=None,
                                op0=AluOpType.add)
                        else:
                            nc.scalar.activation(out=gP[:, h, 0:W],
                                                 in_=a2g[:, 0:W],
                                                 func=AF.Identity,
                                                 bias=bt["a2bg"])
                        g = gP[:, h, 0:W]

                    g_bc3 = bass.AP(tensor=g.tensor, offset=g.offset,
                                    ap=[g.ap[0], [0, 3], g.ap[1]])
                    vout = wkv.tile([F, 3, BLK], BF16, tag="vout")
                    if ABLATE in ("no_gate", "pe_only"):
                        nc.vector.memset(vout, 0.5)
                    elif GATE_MODE == "bcast":
                        nc.vector.tensor_tensor(out=vout[:, :, 0:W],
                                                in0=v13[:, :, 0:W],
                                                in1=g_bc3,
                                                op=AluOpType.mult)
                    else:
                        for c in range(3):
                            nc.vector.tensor_tensor(out=vout[:, c, 0:W],
                                                    in0=v13[:, c, 0:W],
                                                    in1=g,
                                                    op=AluOpType.mult)
                    vouts.append(vout)

                cx["h1P"], cx["vouts"] = h1P, vouts

            def st_mid2(it):
                cx = ctxs[it]
                nh = cx["nh"]
                if ABLATE == "no_mid":
                    n2P2 = wkx.tile([F, nh, BLK], BF16, tag="n2P_l2")
                    nc.vector.memset(n2P2, 0.5)
                    cx["n2P2"] = n2P2
                    return
                _, n2P2 = norm_front(cx.pop("vouts"), nh, "l2", False,
                                     W=cx["W"])
                cx["n2P2"] = n2P2

            def st_tail(it):
                cx = ctxs.pop(it)
                nh, xm, h1P, n2P2 = cx["nh"], cx["xm"], cx["h1P"], cx["n2P2"]
                W = cx["W"]
                h2P = wk.tile([F, nh, BLK], BF16, tag="h2P")
                for h in range(nh):
                    a1b_ = psa.tile([F, BLK], F32, tag="a")
                    nc.tensor.matmul(a1b_[:, 0:W], wt["wp"], h1P[:, h, 0:W],
                                     start=True, stop=False)
                    nc.tensor.matmul(a1b_[:, 0:W], wt["a1w_n1"],
                                     n2P2[:, h, 0:W],
                                     start=False, stop=True)
                    nc.scalar.activation(out=h2P[:, h, 0:W],
                                         in_=a1b_[:, 0:W],
                                         func=AF.Silu, bias=bt["b2p"])

                if ABLATE in ("no_scy",):
                    return
                sc_ps = psa.tile([F, nh, BLK // F, 2], F32, tag="a")
                for h in range(nh):
                    for k in range(W // F):
                        nc.tensor.matmul(sc_ps[:, h, k, :],
                                         h2P[:, h, k * F:(k + 1) * F], wft)
                sc_sb = wk.tile([F, nh, BLK // F, 2], BF16, tag="sc_sb")
                nc.vector.tensor_scalar(out=sc_sb[:, :, 0:W // F, :],
                                        in0=sc_ps[:, :, 0:W // F, :],
                                        scalar1=bt["bfold"],
                                        scalar2=None, op0=AluOpType.add)
                y_ps = psa.tile([F, 2], F32, tag="a")
                nmm = 0
                nmm_tot = nh * (W // F)
                for h in range(nh):
                    mT_t = xm[:, h, 0:W].rearrange("p (k g) -> p k g",
                                                   k=W // F)
                    for k in range(W // F):
                        nc.tensor.matmul(y_ps, mT_t[:, k, :],
                                         sc_sb[:, h, k, :],
                                         start=(nmm == 0),
                                         stop=(nmm == nmm_tot - 1),
                                         skip_group_check=True)
                        nmm += 1
                nc.vector.tensor_tensor(out=y_sb, in0=y_sb, in1=y_ps[:, 0:1],
                                        op=AluOpType.add)

            o1, o2, o3 = OFFSETS
            total = len(groups) * reps
            for it in range(total + PREFETCH + o3):
                if it < total:
                    st_dma(it)
                if 0 <= it - PREFETCH - o3 < total:
                    st_tail(it - PREFETCH - o3)
                if 0 <= it - PREFETCH - o2 < total:
                    st_mid2(it - PREFETCH - o2)
                # front before mid: the psv "v3" ring then recycles buffers
                # whose readers are the fast square/SQA ops (front/mid2)
                # rather than the slow same-iteration DVE gate, so the PE
                # feeds ACT's squares (the bottleneck engine) sooner.
                if 0 <= it - PREFETCH < total:
                    st_front(it - PREFETCH)
                if 0 <= it - PREFETCH - o1 < total:
                    st_mid(it - PREFETCH - o1)

            nc.sync.dma_start(out=y[:, :], in_=y_sb)

    nc.finalize()
    return nc


def kernel(s, v, r, batch_mask, w1, w2, a1w, a1b, a2w, a2b, out_w, out_b):
    global _last_results, GATE_G_FROM_PSUM
    del r  # unused by the reference computation

    # The PSUM-direct gate skips the +a2bg bias (zero for this model init);
    # fall back to the ACT-evac path if a nonzero gate bias ever shows up.
    if np.any(np.asarray(a2b)[0, F:] != 0):
        GATE_G_FROM_PSUM = False

    s = np.ascontiguousarray(np.asarray(s, dtype=np.float32)).reshape(NA_FULL, F)
    v = np.ascontiguousarray(np.asarray(v, dtype=np.float32)).reshape(NA_FULL, 3, F)
    batch_mask = np.ascontiguousarray(
        np.asarray(batch_mask, dtype=np.float32)).reshape(F, NA_FULL)
    w1 = np.asarray(w1, dtype=np.float64)
    w2 = np.asarray(w2, dtype=np.float64)
    a1w = np.asarray(a1w, dtype=np.float64)
    a1b = np.asarray(a1b, dtype=np.float64)
    a2w = np.asarray(a2w, dtype=np.float64)
    a2b = np.asarray(a2b, dtype=np.float64)
    out_w = np.asarray(out_w, dtype=np.float64)
    out_b = np.asarray(out_b, dtype=np.float64)
    assert w1.shape == (2, F, F), "kernel is specialized to L=2"

    bf16 = mybir.dt.np(BF16)

    # folded final projection: sc = h2 @ (a2w_s1 @ out_w) + b_fold
    w_fold = np.zeros((F, 2), dtype=np.float64)
    w_fold[:, 0:1] = a2w[1][:, :F] @ out_w
    b_fold = float(a2b[1][:F] @ out_w[:, 0] + out_b[0])

    # folded layer-2 scalar path: a1w_s1^T s1 = (a2w_s0 @ a1w_s1)^T h1 + const
    wp = a2w[0][:, :F] @ a1w[1][:F, :]
    b2p = a1b[1] + a1w[1][:F, :].T @ a2b[0][:F]

    weights = {
        "w1_0": w1[0], "w2_0": w2[0], "w2_1": w2[1],
        "a1w_s0": a1w[0][:F, :], "a1w_n0": a1w[0][F:, :],
        "a1w_n1": a1w[1][F:, :], "a2w_g0": a2w[0][:, F:], "wp": wp,
    }
    weights = {k: np.ascontiguousarray(a, dtype=bf16)
               for k, a in weights.items()}
    w_fold16 = np.ascontiguousarray(w_fold, dtype=bf16)
    bias_cols = {
        "bfold": np.full((F, 1), b_fold, dtype=np.float32),
        "a1b0": np.ascontiguousarray(a1b[0].reshape(F, 1), dtype=np.float32),
        "b2p": np.ascontiguousarray(b2p.reshape(F, 1), dtype=np.float32),
        "a2bg": np.ascontiguousarray(a2b[0][F:].reshape(F, 1),
                                     dtype=np.float32),
    }

    in_maps = []
    for c in range(N_CORES):
        sl = slice(c * NA_CORE, (c + 1) * NA_CORE)
        sT = np.zeros((F, NA), dtype=np.float32)
        sT[:, :NA_CORE] = s[sl].T
        vT = np.zeros((F, 3, NA), dtype=np.float32)
        vT[:, :, :NA_CORE] = v[sl].transpose(2, 1, 0)
        mT = np.zeros((NA, F), dtype=np.float32)
        mT[:NA_CORE] = batch_mask[:, sl].T
        xp = np.zeros((F, NBLK, 5 * BLK), dtype=np.float32)
        NF = (NBLK - 1) * BLK
        xp[:, :-1, 0:BLK] = sT[:, :NF].reshape(F, NBLK - 1, BLK)
        xp[:, :-1, BLK:4 * BLK] = (
            vT[:, :, :NF].reshape(F, 3, NBLK - 1, BLK).transpose(0, 2, 1, 3)
            .reshape(F, NBLK - 1, 3 * BLK))
        xp[:, :-1, 4 * BLK:] = (
            mT[:NF].reshape(NBLK - 1, BLK // F, F, F).transpose(2, 0, 1, 3)
            .reshape(F, NBLK - 1, BLK))
        # tail block packed tight at TBLK: [s | v0 v1 v2 | mask chunks]
        xp[:, -1, 0:TBLK] = sT[:, NF:NF + TBLK]
        xp[:, -1, TBLK:4 * TBLK] = (
            vT[:, :, NF:NF + TBLK].reshape(F, 3 * TBLK))
        xp[:, -1, 4 * TBLK:5 * TBLK] = (
            mT[NF:NF + TBLK].reshape(TBLK // F, F, F).transpose(1, 0, 2)
            .reshape(F, TBLK))
        m = {"xpack": np.ascontiguousarray(xp, dtype=bf16),
             "w_fold": w_fold16}
        m.update(weights)
        m.update(bias_cols)
        in_maps.append(m)

    nc = _build(b_fold)
    res = run_bass_kernel_spmd(nc, in_maps, core_ids=list(range(N_CORES)))
    global _last_nc, _last_in_maps
    _last_results, _last_nc, _last_in_maps = res, nc, in_maps

    yv = np.zeros((F, 1), dtype=np.float64)
    for c in range(N_CORES):
        yv += res.results[c]["y"].astype(np.float64)
    return yv.astype(np.float32)

